# revision 1
# baseline (speedup 1.0000x reference)
"""Contrastive-loss kernel for Trainium2, SPMD over 8 NeuronCores.

The reference loss over x[N=4, S=4096, F=256] is, for pairs a>b with
D[a,b] = ||x[:,a]-x[:,b]||^2 (summed over batch and feature):

    loss = [ sum_{a>b, a-b>1} D[a,b] + sum_{b} relu(M - D[b+1,b]) ] / (S*(S-1)*1000)

Using symmetry of D (zero diagonal) this collapses to a streaming
computation that never materializes the [S,S] Gram matrix:

    sum_{a>b} D       = S * sum_t s[t] - sum_{n,f} c[n,f]^2
    s[t]              = sum_{n,f} x[n,t,f]^2
    c[n,f]            = sum_t x[n,t,f]
    D_sub[b]          = s[b] + s[b+1] - 2 * sum_{n,f} x[n,b+1,f]*x[n,b,f]
    numerator         = sum_{a>b} D - sum_b D_sub[b] + sum_b relu(M - D_sub[b])

Sharding: split the sequence dim into 8 chunks of 512 positions (+1 halo
row for the shifted product). Each core computes its partial c[1024],
sum_s, sum_D_sub and sum_hinge fully on device; the host gathers the 8
partial vectors/scalars and combines them (a ~1k-element sum of squares
plus a handful of adds) in float64.
"""

import numpy as np

import concourse.bass as bass
import concourse.tile as tile
from concourse import mybir
from concourse.bass_utils import run_bass_kernel_spmd

N, S, F = 4, 4096, 256
NCORES = 8
LOCAL = S // NCORES            # 512 positions per core
CH = LOCAL + 1                 # 513 chunk rows (1-row halo)
NBLK = LOCAL // 128            # 4 partition-blocks per core
NF = N * F                     # 1024
MARGIN = 60000.0
OUT_W = NF + 3                 # c partial (1024) + [sum_s, sum_D, sum_hinge]

_program = None
TRACE = False
LAST_RESULT = None


def _patch_sem_clear():
    """The walrus build in this container cannot encode
    EVENT_SEMAPHORE_RANGE_CLEAR ("ISA wrong length" in codegen). Replace the
    tail range-clear that TileContext emits via Bass.clear_and_free_semaphores
    with per-semaphore EventSemaphore writes of 0 (sem-wr-imm), which the
    compiler does support."""
    import bass_rust
    from concourse.bass import compact_to_ranges

    if getattr(bass.Bass, "_sem_clear_patched", False):
        return

    def clear_and_free_semaphores(self, sems):
        if not sems:
            return
        sem_nums = [s.num if hasattr(s, "num") else s for s in sems]
        for sem_range in compact_to_ranges(sem_nums):
            assert self._state.free_isdisjoint(sem_range)
            self.gpsimd.dma_reset(sem_range)
            for num in sem_range:
                h = bass_rust.SemaphoreHandle(num=num, name=f"clr{num}")
                bi = self.gpsimd.sem_inc(h, 1)
                upd = bi.ins.sync_info.on_update[0]
                upd.update_mode = "sem-wr-imm"
                upd.update_value = 0
        self._state.prepend_free_semaphores(sem_nums)
        for poison_set in self._tile_sem_poison_stack:
            poison_set.update(sem_nums)

    bass.Bass.clear_and_free_semaphores = clear_and_free_semaphores
    bass.Bass._sem_clear_patched = True


def _split_multi_waits(nc: bass.Bass) -> None:
    """The walrus build here encodes at most ONE sync wait per instruction.
    Hoist surplus waits into standalone wait-only EventSemaphore instructions
    placed immediately before the owner on the same engine queue — semantics
    are identical (same queue, in-order), and every instruction ends up with
    a single wait."""
    import bass_rust

    wid = 0
    for b in nc.m.functions[0].blocks:
        out = []
        changed = False
        for inst in b.instructions:
            si = inst.sync_info
            waits = list(si.on_wait) if si is not None else []
            if len(waits) > 1:
                changed = True
                for w in waits[:-1]:
                    ev = bass_rust.InstEventSemaphore(
                        name=f"WSPLIT-{wid}", engine=inst.engine, ins=[], outs=[]
                    )
                    wid += 1
                    ev.sync_info = bass_rust.SyncInfo(on_wait=[w], on_update=[])
                    out.append(ev)
                inst.sync_info = bass_rust.SyncInfo(
                    on_wait=[waits[-1]], on_update=list(si.on_update)
                )
            out.append(inst)
        if changed:
            b.instructions = out


def _build_program() -> bass.Bass:
    _patch_sem_clear()
    f32 = mybir.dt.float32
    nc = bass.Bass()
    xc = nc.declare_dram_parameter("xc", [N, CH, F], f32, isOutput=False)
    msk = nc.declare_dram_parameter("mask", [128, NBLK], f32, isOutput=False)
    out = nc.declare_dram_parameter("out", [1, NF], f32, isOutput=True)
    # out2[0, :]   = unmasked partition sums of [sA cols | D cols | hinge cols]
    # out2[1+b, :] = mask-col-b weighted partition sums of the same
    out2 = nc.declare_dram_parameter("out2", [1 + NBLK, 3 * NBLK], f32, isOutput=True)

    with tile.TileContext(nc) as tc:
        with (
            tc.tile_pool(name="data", bufs=4) as data,
            tc.tile_pool(name="scratch", bufs=4) as scratch,
            tc.tile_pool(name="small", bufs=1) as small,
            tc.tile_pool(name="psum", bufs=1, space="PSUM") as psum,
        ):
            # Issue the block loads before anything else: descriptor
            # generation for 512 descriptors takes ~3us and gates the first
            # byte. Alternate between the two HWDGE rings (SP + ACT) so two
            # generators run in parallel.
            # AB[p, n, 0:F] = x[n, r0+p, :], AB[p, n, F:2F] = x[n, r0+p+1, :]
            # (consecutive rows are contiguous in DRAM: one 2F-span each).
            xc_base = xc[:, :, :]
            ABs = []
            for blk in range(NBLK):
                r0 = blk * 128
                AB = data.tile([128, N, 2 * F], f32, tag="AB")
                src = bass.AP(
                    tensor=xc_base.tensor,
                    offset=r0 * F,
                    ap=[[F, 128], [CH * F, N], [1, 2 * F]],
                )
                eng = nc.sync if blk % 2 == 0 else nc.scalar
                eng.dma_start(out=AB, in_=src)
                ABs.append(AB)

            # lhsT for the final masked reduction: [ones | mask col 0..3]
            onesmask = small.tile([128, 1 + NBLK], f32)
            nc.vector.memset(onesmask[:, 0:1], 1.0)
            onesb = small.tile([128, 1], mybir.dt.bfloat16)
            nc.vector.memset(onesb, 1.0)
            nc.sync.dma_start(out=onesmask[:, 1 : 1 + NBLK], in_=msk[:, :])
            marg = small.tile([128, 1], f32)
            nc.vector.memset(marg, MARGIN)
            # Warm the Relu table while ACT waits for data, so the late
            # hinge op doesn't pay the table load on the critical path.
            warm = small.tile([128, 1], f32)
            nc.scalar.activation(
                out=warm,
                in_=marg,
                func=mybir.ActivationFunctionType.Relu,
                bias=marg[:, 0:1],
                scale=-1.0,
            )

            sA = small.tile([128, NBLK], f32)       # per-position s[t]
            Dh = small.tile([128, 2 * NBLK], f32)   # [D cols | hinge cols], ACT-written

            pc0 = psum.tile([1, 512], f32)          # c[:512] accumulator
            pc1 = psum.tile([1, 512], f32)          # c[512:] accumulator
            # [ones | mask-col-b] weighted partition sums of
            # [sA cols | D cols | hinge cols]
            pfin = psum.tile([1 + NBLK, 3 * NBLK], f32)

            for blk in range(NBLK):
                AB = ABs[blk]
                A = AB[:, :, 0:F]
                B = AB[:, :, F : 2 * F]

                diff = scratch.tile([128, N, F], f32, tag="diff")
                sqA = scratch.tile([128, N, F], f32, tag="sqA")
                sqd = scratch.tile([128, N, F], f32, tag="sqd")

                # D_sub[t] = ||row_{t+1} - row_t||^2 via one DVE sub and one
                # fused square+accumulate on the scalar engine.
                nc.vector.tensor_sub(diff, A, B)
                nc.scalar.activation(
                    out=sqd,
                    in_=diff,
                    func=mybir.ActivationFunctionType.Square,
                    accum_out=Dh[:, blk : blk + 1],
                )
                # s[t] via the scalar engine's fused square+accumulate
                # (one pass, no separate reduce).
                nc.scalar.activation(
                    out=sqA,
                    in_=A,
                    func=mybir.ActivationFunctionType.Square,
                    accum_out=sA[:, blk : blk + 1],
                )

                # per-block hinge col: relu(MARGIN - D), same engine as the
                # D write so only one short ACT hop per block.
                nc.scalar.activation(
                    out=Dh[:, NBLK + blk : NBLK + blk + 1],
                    in_=Dh[:, blk : blk + 1],
                    func=mybir.ActivationFunctionType.Relu,
                    bias=marg[:, 0:1],
                    scale=-1.0,
                )

                # f32 matmuls run at 1/4 PE rate; cast A to bf16 on the
                # (underutilized) DVE so the column-sum matmuls go 4x
                # faster. c only feeds the small sum(c^2) correction term,
                # so bf16 costs ~1e-6 relative on the final loss.
                Abf = scratch.tile([128, N, F], mybir.dt.bfloat16, tag="abf")
                nc.vector.tensor_copy(Abf, A)
                first, last = blk == 0, blk == NBLK - 1
                nc.tensor.matmul(pc0, onesb, Abf[:, 0:2, :], start=first, stop=last)
                nc.tensor.matmul(pc1, onesb, Abf[:, 2:4, :], start=first, stop=last)

                # Per-block final reduction columns: blocks 0-2 finish in the
                # DMA shadow; only block 3's three tiny matmuls trail.
                nc.tensor.matmul(
                    pfin[:, blk : blk + 1],
                    onesmask,
                    sA[:, blk : blk + 1],
                    start=True,
                    stop=True,
                )
                nc.tensor.matmul(
                    pfin[:, NBLK + blk : NBLK + blk + 1],
                    onesmask,
                    Dh[:, blk : blk + 1],
                    start=True,
                    stop=True,
                )
                nc.tensor.matmul(
                    pfin[:, 2 * NBLK + blk : 2 * NBLK + blk + 1],
                    onesmask,
                    Dh[:, NBLK + blk : NBLK + blk + 1],
                    start=True,
                    stop=True,
                )

            # PSUM -> SBUF staging on the (idle-by-now) DVE, keeping the
            # scalar engine's tail chain short.
            ob = small.tile([1, NF], f32)
            nc.vector.tensor_copy(ob[:, 0:512], pc0)
            nc.vector.tensor_copy(ob[:, 512:1024], pc1)
            ob2 = small.tile([1 + NBLK, 3 * NBLK], f32)
            nc.vector.tensor_copy(ob2, pfin)
            # Separate rings: the c-vector store must not queue behind the
            # later-arriving pfin store's waits (FIFO head-of-line).
            nc.sync.dma_start(out=out[:, :], in_=ob)
            nc.scalar.dma_start(out=out2[:, :], in_=ob2)
    _split_multi_waits(nc)
    return nc


def _get_program() -> bass.Bass:
    global _program
    if _program is None:
        _program = _build_program()
    return _program


def kernel(**inputs) -> np.ndarray:
    global LAST_RESULT
    x = np.ascontiguousarray(np.asarray(inputs["x"], dtype=np.float32))
    assert x.shape == (N, S, F)
    nc = _get_program()

    in_maps = []
    for k in range(NCORES):
        t0 = k * LOCAL
        take = min(CH, S - t0)
        chunk = np.zeros((N, CH, F), dtype=np.float32)
        chunk[:, :take, :] = x[:, t0 : t0 + take, :]
        m = np.ones((128, NBLK), dtype=np.float32)
        if k == NCORES - 1:
            m[127, NBLK - 1] = 0.0
        in_maps.append({"xc": chunk, "mask": m})

    LAST_RESULT = run_bass_kernel_spmd(
        nc, in_maps, list(range(NCORES)), trace=TRACE
    )
    res = LAST_RESULT.results

    c = np.zeros(NF, dtype=np.float64)
    ssum = dsum = hsum = 0.0
    for r in res:
        c += r["out"][0].astype(np.float64)
        o2 = r["out2"].astype(np.float64)
        # row 0: unmasked sums; row 1+b: mask-col-b sums.
        # cols 0:NBLK = sA, NBLK:2N = D, 2N:3N = hinge
        ssum += o2[0, 0:NBLK].sum()
        for b in range(NBLK):
            dsum += o2[1 + b, NBLK + b]
            hsum += o2[1 + b, 2 * NBLK + b]
    gsum = float(np.sum(c * c))
    numerator = S * ssum - gsum - dsum + hsum
    loss = numerator / float(S * (S - 1) * 1000)
    return np.asarray(loss, dtype=np.float32)

